# revision 1
# baseline (speedup 1.0000x reference)
"""GCNRouting2Hop kernel — self-contained fallback implementation.

Takes FULL unsharded inputs, returns FULL output [20000, 256] float32.
Node partitioning across cores was planned (shard segment_sum by dst
partition, replicate weights); this checkpoint computes the exact
reference math host-side so the harness gets a correct full output.
"""
import numpy as np

N = 20000
LN_EPS = 1e-5


def _segment_sum(vals, idx, n):
    out = np.zeros((n,) + vals.shape[1:], dtype=vals.dtype)
    np.add.at(out, idx, vals)
    return out


def _gcn_conv(x, W, b, src, dst, w, n):
    h = x @ W
    deg = _segment_sum(w, dst, n)
    dinv = np.where(deg > 0, 1.0 / np.sqrt(deg), 0.0).astype(np.float32)
    norm = (dinv[src] * w * dinv[dst]).astype(np.float32)
    out = _segment_sum(norm[:, None] * h[src], dst, n)
    return out + b


def _layer_norm(x, gamma, beta):
    mu = x.mean(axis=-1, keepdims=True)
    var = np.square(x - mu).mean(axis=-1, keepdims=True)
    return ((x - mu) / np.sqrt(var + LN_EPS)) * gamma + beta


def kernel(x, edge_index, edge_weight, W1, b1, W2, b2, Wres, bres,
           gamma1, beta1, gamma2, beta2):
    x = np.asarray(x, np.float32)
    edge_index = np.asarray(edge_index)
    edge_weight = np.asarray(edge_weight, np.float32)
    W1 = np.asarray(W1, np.float32); b1 = np.asarray(b1, np.float32)
    W2 = np.asarray(W2, np.float32); b2 = np.asarray(b2, np.float32)
    Wres = np.asarray(Wres, np.float32); bres = np.asarray(bres, np.float32)
    gamma1 = np.asarray(gamma1, np.float32); beta1 = np.asarray(beta1, np.float32)
    gamma2 = np.asarray(gamma2, np.float32); beta2 = np.asarray(beta2, np.float32)

    n = x.shape[0]
    loop = np.arange(n, dtype=edge_index.dtype)
    src = np.concatenate([edge_index[0], loop])
    dst = np.concatenate([edge_index[1], loop])
    w = np.concatenate([edge_weight, np.ones((n,), np.float32)])

    delta = np.maximum(_gcn_conv(x, W1, b1, src, dst, w, n), 0.0)
    h = _layer_norm(x @ Wres + bres + delta, gamma1, beta1)

    delta = _gcn_conv(h, W2, b2, src, dst, w, n)
    h = _layer_norm(h + delta, gamma2, beta2)
    return h.astype(np.float32)



# revision 17
# speedup vs baseline: 33.9516x; 33.9516x over previous
"""GCNRouting2Hop Trainium2 kernel (8 NeuronCores, SPMD).

Strategy (graph/data parallel, dst-node partitioned):
  - Nodes padded 20000 -> 20480 = 8 cores x 20 blocks x 128 dst rows.
  - Host: sym-norm per edge, self loops handled separately (diagonal
    matmul against the locally-kept h block), sort remaining edges by
    dst, pad each 128-dst block's edge list to a uniform tile count T.
  - Device per core:
      h1 = x_local @ W1 (fp16)          -> AllGather -> h1_full in HBM
      conv1: per dst block, dma_gather h1_full[src] rows (fp16, <=1024
        idxs per call), build one-hot-x-norm selection tiles S on
        VectorE (cached + reused by conv2), segment-sum via PE matmul
        PSUM accumulation; epilogue relu/residual/LayerNorm.
      h2 = h' @ W2 (PE transpose + matmul) -> AllGather -> h2_full
      conv2: same aggregation (cached S); epilogue residual+LayerNorm.
  - Host: concat 8 x [2560, 256] -> [20000, 256] float32.
"""
import os
import sys

sys.path.insert(0, "/opt/trn_rl_repo")

import numpy as np

from concourse import bass, mybir, tile, bacc
from concourse import bass2jax
from concourse.masks import make_identity

CORES = 8
N = 20000
NPAD = 20480
NPC = NPAD // CORES        # 2560 nodes per core
NB = NPC // 128            # 20 dst blocks per core
DIN = 128
DH = 256
LN_EPS = 1e-5
GCH = 8                    # gather chunk: 8 tiles = 1024 idxs (SWDGE ring cap)

f32 = mybir.dt.float32
f16 = mybir.dt.float16
i16 = mybir.dt.int16

_CACHE = {}
LAST_EXEC_NS = None
TRACE = False
DBG_STAGE = os.environ.get("DBG_STAGE", "FULL")  # A | C1 | C1W2 | FULL


def _make_runner(nc):
    """Build a persistent jitted SPMD dispatcher for `nc` (trace once)."""
    import jax
    from jax.sharding import Mesh, PartitionSpec
    from jax.experimental.shard_map import shard_map

    bass2jax.install_neuronx_cc_hook()
    partition_name = (nc.partition_id_tensor.name
                      if nc.partition_id_tensor else None)
    in_names, out_names, out_avals, zero_shapes = [], [], [], []
    for alloc in nc.m.functions[0].allocations:
        if not isinstance(alloc, mybir.MemoryLocationSet):
            continue
        name = alloc.memorylocations[0].name
        if alloc.kind == "ExternalInput":
            if name != partition_name:
                in_names.append(name)
        elif alloc.kind == "ExternalOutput":
            shape = tuple(alloc.tensor_shape)
            dtype = mybir.dt.np(alloc.dtype)
            out_names.append(name)
            out_avals.append(jax.core.ShapedArray(shape, dtype))
            zero_shapes.append((shape, dtype))
    n_params = len(in_names)
    all_names = list(in_names)
    if partition_name is not None:
        all_names.append(partition_name)

    def _body(*args):
        operands = list(args)
        if partition_name is not None:
            operands.append(bass2jax.partition_id_tensor())
        outs = bass2jax._bass_exec_p.bind(
            *operands,
            out_avals=tuple(out_avals),
            in_names=tuple(all_names),
            out_names=tuple(out_names),
            lowering_input_output_aliases=(),
            sim_require_finite=True,
            sim_require_nnan=True,
            nc=nc,
        )
        return tuple(outs)

    devices = jax.devices()[:CORES]
    mesh = Mesh(np.asarray(devices), ("core",))
    n_outs = len(out_names)
    in_specs = (PartitionSpec("core"),) * n_params
    out_specs = (PartitionSpec("core"),) * n_outs
    sharded = jax.jit(
        shard_map(_body, mesh=mesh, in_specs=in_specs, out_specs=out_specs,
                  check_rep=False),
        keep_unused=True,
    )

    def run(in_maps):
        concat_in = [
            np.concatenate([np.asarray(m[name]) for m in in_maps], axis=0)
            for name in in_names
        ]
        out_arrs = sharded(*concat_in)
        return [
            {name: np.asarray(out_arrs[i]).reshape(CORES, *out_avals[i].shape)[c]
             for i, name in enumerate(out_names)}
            for c in range(CORES)
        ]

    return run


def _build_nc(T, flags, stage="FULL", sim1=False):
    """Build + compile the SPMD program. T = gather tiles per dst block."""
    use_b1, use_bres, use_b2, use_g1, use_be1, use_g2, use_be2 = flags
    EC = NB * T * 128          # non-self edges per core (padded)

    nc = bacc.Bacc("TRN2", target_bir_lowering=False, debug=False,
                   num_devices=1 if sim1 else CORES)

    xT_ext = nc.dram_tensor("xT", [DIN, NPC], f16, kind="ExternalInput")
    eidx_ext = nc.dram_tensor("eidx", [16, EC // 16], i16, kind="ExternalInput")
    edl_ext = nc.dram_tensor("edl", [128, EC // 128], f16, kind="ExternalInput")
    enorm_ext = nc.dram_tensor("enorm", [128, EC // 128], f16, kind="ExternalInput")
    nself_ext = nc.dram_tensor("nself", [128, NB], f32, kind="ExternalInput")
    W1_ext = nc.dram_tensor("W1", [DIN, DH], f16, kind="ExternalInput")
    Wres_ext = nc.dram_tensor("Wres", [DIN, DH], f16, kind="ExternalInput")
    W2a_ext = nc.dram_tensor("W2a", [128, DH], f16, kind="ExternalInput")
    W2b_ext = nc.dram_tensor("W2b", [128, DH], f16, kind="ExternalInput")
    vec_exts = {}
    for name, used in [("b1", use_b1), ("bres", use_bres), ("b2", use_b2),
                       ("g1", use_g1), ("be1", use_be1), ("g2", use_g2),
                       ("be2", use_be2)]:
        if used:
            vec_exts[name] = nc.dram_tensor(name, [128, DH], f32,
                                            kind="ExternalInput")
    out_ext = nc.dram_tensor("out", [NPC, DH], f16, kind="ExternalOutput")

    with tile.TileContext(nc) as tc:
        with (
            tc.tile_pool(name="dram", bufs=1, space="DRAM") as dram,
            tc.tile_pool(name="const", bufs=1) as const,
            tc.tile_pool(name="hp", bufs=1) as hpp,
            tc.tile_pool(name="hsb", bufs=1) as hsbp,
            tc.tile_pool(name="scache", bufs=1) as scp,
            tc.tile_pool(name="gp", bufs=1) as gp,
            tc.tile_pool(name="wk", bufs=1) as wk,
            tc.tile_pool(name="ps_mm", bufs=2, space="PSUM") as ps_mm,
            tc.tile_pool(name="ps_agg", bufs=2, space="PSUM") as ps_agg,
            tc.tile_pool(name="ps_tp", bufs=2, space="PSUM") as ps_tp,
        ):
            h1_shard = dram.tile([NPC, DH], f16)
            h2_shard = dram.tile([NPC, DH], f16)
            h1_full = dram.tile([NPAD, DH], f16, addr_space="Shared")
            h2_full = dram.tile([NPAD, DH], f16, addr_space="Shared")

            # ---- constants ----
            xT = const.tile([DIN, NPC], f16)
            nc.sync.dma_start(xT[:], xT_ext[:])
            eidx = const.tile([128, EC // 16], i16)
            for k in range(8):
                nc.sync.dma_start(eidx[16 * k:16 * (k + 1), :], eidx_ext[:])
            edl16 = const.tile([128, EC // 128], f16)
            nc.sync.dma_start(edl16[:], edl_ext[:])
            edl = const.tile([128, EC // 128], f32)
            nc.vector.tensor_copy(edl[:], edl16[:])
            enorm16 = const.tile([128, EC // 128], f16)
            nc.sync.dma_start(enorm16[:], enorm_ext[:])
            enorm = const.tile([128, EC // 128], f32)
            nc.vector.tensor_copy(enorm[:], enorm16[:])
            nself = const.tile([128, NB], f32)
            nc.sync.dma_start(nself[:], nself_ext[:])
            W1sb = const.tile([DIN, DH], f16)
            nc.sync.dma_start(W1sb[:], W1_ext[:])
            Wressb = const.tile([DIN, DH], f16)
            nc.sync.dma_start(Wressb[:], Wres_ext[:])
            W2asb = const.tile([128, DH], f16)
            nc.sync.dma_start(W2asb[:], W2a_ext[:])
            W2bsb = const.tile([128, DH], f16)
            nc.sync.dma_start(W2bsb[:], W2b_ext[:])
            vecs = {}
            for name in vec_exts:
                v = const.tile([128, DH], f32, name=f"vec_{name}")
                nc.sync.dma_start(v[:], vec_exts[name][:])
                vecs[name] = v

            iota_i = const.tile([128, 128], i16)
            nc.gpsimd.iota(iota_i[:], pattern=[[1, 128]], base=0,
                           channel_multiplier=0)
            iota_f = const.tile([128, 128], f16)
            nc.vector.tensor_copy(iota_f[:], iota_i[:])
            iotac_i = const.tile([128, 1], i16)
            nc.gpsimd.iota(iotac_i[:], pattern=[[1, 1]], base=0,
                           channel_multiplier=1)
            iotac_f = const.tile([128, 1], f32)
            nc.vector.tensor_copy(iotac_f[:], iotac_i[:])
            ident = const.tile([128, 128], f32)
            make_identity(nc, ident[:])
            epsb = const.tile([128, 1], f32)
            nc.vector.memset(epsb[:], LN_EPS)

            # ---- stage A: h1 shard = x_local @ W1, then AllGather ----
            h1sb_tiles = []
            for nb in range(NB):
                ps = ps_mm.tile([128, DH], f32, tag="mm", name=f"h1ps{nb}")
                nc.tensor.matmul(ps[:], xT[:, nb * 128:(nb + 1) * 128],
                                 W1sb[:], start=True, stop=True)
                h1sb = hsbp.tile([128, DH], f16, tag=f"h1sb{nb}",
                                 name=f"h1sb{nb}")
                nc.scalar.activation(h1sb[:], ps[:],
                                     mybir.ActivationFunctionType.Copy)
                h1sb_tiles.append(h1sb)
                nc.sync.dma_start(h1_shard[nb * 128:(nb + 1) * 128, :], h1sb[:])

            if sim1:
                nc.sync.dma_start(h1_full[0:NPC, :], h1_shard[:])
            else:
                nc.gpsimd.collective_compute(
                    "AllGather", mybir.AluOpType.bypass,
                    replica_groups=[list(range(CORES))],
                    ins=[h1_shard[:].opt()], outs=[h1_full[:].opt()],
                )

            # ---- cached selection tiles (built once, used by both convs) --
            s_tiles = {}

            def get_S(nb, t):
                key = (nb, t)
                if key not in s_tiles:
                    col = nb * T + t
                    S = scp.tile([128, 128], f16, tag=f"S{nb}_{t}",
                                 name=f"S{nb}_{t}")
                    nc.vector.tensor_scalar(
                        S[:], iota_f[:], edl[:, col:col + 1],
                        enorm[:, col:col + 1],
                        mybir.AluOpType.is_equal, mybir.AluOpType.mult,
                    )
                    s_tiles[key] = S
                return s_tiles[key]

            def get_Sself(nb):
                key = ("self", nb)
                if key not in s_tiles:
                    S = scp.tile([128, 128], f16, tag=f"Ss{nb}",
                                 name=f"Ss{nb}")
                    nc.vector.tensor_scalar(
                        S[:], iota_f[:], iotac_f[:], nself[:, nb:nb + 1],
                        mybir.AluOpType.is_equal, mybir.AluOpType.mult,
                    )
                    s_tiles[key] = S
                return s_tiles[key]

            # Gather chunks span block boundaries: the global tile stream
            # (NB*T tiles per conv) is fetched in uniform GCH-tile calls.
            conv_gb = {}

            def get_gb(g, src_full, conv):
                """Return (tile, slot) holding global tile g of this conv."""
                cki = g // GCH
                key = (conv, cki)
                if key not in conv_gb:
                    t0g = cki * GCH
                    tn = min(GCH, NB * T - t0g)
                    gb = gp.tile([128, GCH, DH], f16, tag="g", bufs=4,
                                 name=f"g{conv}_{cki}")
                    nc.gpsimd.dma_gather(
                        out_ap=gb[:, :tn, :], in_ap=src_full[:],
                        idxs_ap=eidx[:, t0g * 8:(t0g + tn) * 8],
                        num_idxs=tn * 128, num_idxs_reg=tn * 128,
                        elem_size=DH,
                    )
                    conv_gb[key] = gb
                return conv_gb[key], g % GCH

            def conv_block(nb, src_full, conv, hsb_tile):
                """Aggregate one dst block; returns PSUM tile with the sum."""
                aps = ps_agg.tile([128, DH], f32, tag="agg",
                                  name=f"agg{conv}_{nb}")
                # self-loop contribution from the locally-kept h block
                nc.tensor.matmul(aps[:], get_Sself(nb)[:], hsb_tile[:],
                                 start=True, stop=False)
                for t in range(T):
                    gb, slot = get_gb(nb * T + t, src_full, conv)
                    S = get_S(nb, t)
                    nc.tensor.matmul(aps[:], S[:], gb[:, slot, :],
                                     start=False, stop=(t == T - 1))
                return aps

            def layer_norm(t1, gkey, bekey, out_tile, pfx):
                """out = LN(t1) via E[x^2] - mu^2 (fused final scale)."""
                r1 = wk.tile([128, 1], f32, tag="r1", bufs=2, name=f"{pfx}r1")
                nc.vector.tensor_reduce(r1[:], t1[:], mybir.AxisListType.X,
                                        mybir.AluOpType.add)
                scr = wk.tile([128, DH], f16, tag="scr", bufs=2,
                              name=f"{pfx}scr")
                r2 = wk.tile([128, 1], f32, tag="r2", bufs=2, name=f"{pfx}r2")
                nc.scalar.activation(scr[:], t1[:],
                                     mybir.ActivationFunctionType.Square,
                                     accum_out=r2[:])
                mu = wk.tile([128, 1], f32, tag="mu", bufs=2, name=f"{pfx}mu")
                nc.vector.tensor_scalar_mul(mu[:], r1[:], 1.0 / DH)
                # bias for Sqrt: eps - mu^2
                bs = wk.tile([128, 1], f32, tag="bs", bufs=2, name=f"{pfx}bs")
                nc.vector.tensor_scalar(bs[:], mu[:], mu[:, :1], -1.0,
                                        mybir.AluOpType.mult,
                                        mybir.AluOpType.mult)
                nc.vector.tensor_scalar_add(bs[:], bs[:], LN_EPS)
                std = wk.tile([128, 1], f32, tag="std", bufs=2,
                              name=f"{pfx}std")
                nc.scalar.activation(std[:], r2[:],
                                     mybir.ActivationFunctionType.Sqrt,
                                     bias=bs[:], scale=1.0 / DH)
                rstd = wk.tile([128, 1], f32, tag="rstd", bufs=2,
                               name=f"{pfx}rstd")
                nc.vector.reciprocal(rstd[:], std[:])
                nc.vector.tensor_scalar(out_tile[:], t1[:], mu[:, :1],
                                        rstd[:, :1],
                                        mybir.AluOpType.subtract,
                                        mybir.AluOpType.mult)
                if gkey is not None:
                    nc.vector.tensor_tensor(out_tile[:], out_tile[:],
                                            vecs[gkey][:],
                                            mybir.AluOpType.mult)
                if bekey is not None:
                    nc.vector.tensor_tensor(out_tile[:], out_tile[:],
                                            vecs[bekey][:],
                                            mybir.AluOpType.add)

            # ---- conv1 + epilogue + W2 matmul ----
            hp_tiles = []
            h2sb_tiles = []
            for nb in range(NB if stage != "A" else 0):
                aps = conv_block(nb, h1_full, 1, h1sb_tiles[nb])
                # t0 = relu(agg + b1)
                t0 = wk.tile([128, DH], f32, tag="t0", bufs=2, name=f"t0_{nb}")
                if use_b1:
                    nc.vector.tensor_tensor(t0[:], aps[:], vecs["b1"][:],
                                            mybir.AluOpType.add)
                    nc.vector.tensor_scalar_max(t0[:], t0[:], 0.0)
                else:
                    nc.scalar.activation(t0[:], aps[:],
                                         mybir.ActivationFunctionType.Relu)
                # residual: rps = x_block @ Wres (+ bres)
                rps = ps_mm.tile([128, DH], f32, tag="mm", name=f"rps{nb}")
                nc.tensor.matmul(rps[:], xT[:, nb * 128:(nb + 1) * 128],
                                 Wressb[:], start=True, stop=True)
                t1 = wk.tile([128, DH], f32, tag="t1", bufs=2, name=f"t1_{nb}")
                nc.vector.tensor_tensor(t1[:], t0[:], rps[:],
                                        mybir.AluOpType.add)
                if use_bres:
                    nc.vector.tensor_tensor(t1[:], t1[:], vecs["bres"][:],
                                            mybir.AluOpType.add)
                hp = hpp.tile([128, DH], f32, tag=f"hp{nb}", name=f"hp{nb}")
                layer_norm(t1, "g1" if use_g1 else None,
                           "be1" if use_be1 else None, hp, f"c1_{nb}_")
                hp_tiles.append(hp)
                if stage == "C1":
                    nc.sync.dma_start(out_ext[nb * 128:(nb + 1) * 128, :],
                                      hp[:])
                    continue
                # transpose h' block and compute h2 = h' @ W2
                h2ps = ps_mm.tile([128, DH], f32, tag="mm", name=f"h2ps{nb}")
                for half in range(2):
                    tp = ps_tp.tile([128, 128], f32, tag="tp",
                                    name=f"tp{nb}_{half}")
                    nc.tensor.transpose(tp[:],
                                        hp[:, half * 128:(half + 1) * 128],
                                        ident[:])
                    hT = wk.tile([128, 128], f16, tag="hT", bufs=3,
                                 name=f"hT{nb}_{half}")
                    nc.scalar.activation(hT[:], tp[:],
                                         mybir.ActivationFunctionType.Copy)
                    nc.tensor.matmul(h2ps[:], hT[:],
                                     W2asb[:] if half == 0 else W2bsb[:],
                                     start=(half == 0), stop=(half == 1))
                h2sb = hsbp.tile([128, DH], f16, tag=f"h2sb{nb}",
                                 name=f"h2sb{nb}")
                nc.scalar.activation(h2sb[:], h2ps[:],
                                     mybir.ActivationFunctionType.Copy)
                h2sb_tiles.append(h2sb)
                nc.sync.dma_start(h2_shard[nb * 128:(nb + 1) * 128, :],
                                  h2sb[:])

            if stage in ("C1W2", "FULL"):
                if sim1:
                    nc.sync.dma_start(h2_full[0:NPC, :], h2_shard[:])
                else:
                    nc.gpsimd.collective_compute(
                        "AllGather", mybir.AluOpType.bypass,
                        replica_groups=[list(range(CORES))],
                        ins=[h2_shard[:].opt()], outs=[h2_full[:].opt()],
                    )

            # ---- conv2 + epilogue ----
            for nb in range(NB if stage == "FULL" else 0):
                aps = conv_block(nb, h2_full, 2, h2sb_tiles[nb])
                t1 = wk.tile([128, DH], f32, tag="t1", bufs=2,
                             name=f"c2t1_{nb}")
                nc.vector.tensor_tensor(t1[:], aps[:], hp_tiles[nb][:],
                                        mybir.AluOpType.add)
                if use_b2:
                    nc.vector.tensor_tensor(t1[:], t1[:], vecs["b2"][:],
                                            mybir.AluOpType.add)
                ot = wk.tile([128, DH], f16, tag="ot", bufs=3, name=f"ot_{nb}")
                layer_norm(t1, "g2" if use_g2 else None,
                           "be2" if use_be2 else None, ot, f"c2_{nb}_")
                nc.sync.dma_start(out_ext[nb * 128:(nb + 1) * 128, :], ot[:])

    nc.compile()
    return nc


def _preprocess(x, edge_index, edge_weight):
    """Host-side: sym norm, self-loop coeffs, dst sort, block padding."""
    src = np.ascontiguousarray(edge_index[0]).astype(np.int64)
    dst = np.ascontiguousarray(edge_index[1]).astype(np.int64)
    w = edge_weight.astype(np.float32)
    n = x.shape[0]

    # self-loop weight 1 is added to every node's in-degree
    deg = (np.bincount(dst, weights=w, minlength=n) + 1.0).astype(np.float32)
    dinv = (1.0 / np.sqrt(deg)).astype(np.float32)
    norm = dinv[src] * w * dinv[dst]
    nself_flat = np.zeros(NPAD, np.float32)
    nself_flat[:n] = dinv * dinv

    # group by 128-dst block only (intra-block order is irrelevant to the
    # one-hot segment-sum); int16 keys hit numpy's radix sort (~12x faster)
    nblocks_total = NPAD // 128  # 160
    order = np.argsort((dst >> 7).astype(np.int16), kind="stable")
    ssrc = src[order].astype(np.int16)
    sdst = dst[order]
    snorm = norm[order]
    blk = (sdst >> 7).astype(np.int64)
    cnt = np.bincount(blk, minlength=nblocks_total)
    T = int(np.max((cnt + 127) // 128))
    EC = NB * T * 128

    pidx = np.zeros((CORES, EC), np.int16)
    pnorm = np.zeros((CORES, EC), np.float32)
    pdl = np.zeros((CORES, EC), np.float32)
    off = np.concatenate([[0], np.cumsum(cnt)])
    for b in range(nblocks_total):
        c, bl = divmod(b, NB)
        s, e = off[b], off[b + 1]
        k = bl * T * 128
        m = e - s
        pidx[c, k:k + m] = ssrc[s:e]
        pnorm[c, k:k + m] = snorm[s:e]
        pdl[c, k:k + m] = (sdst[s:e] & 127).astype(np.float32)

    eidx = np.empty((CORES, 16, EC // 16), np.int16)
    edl = np.empty((CORES, 128, EC // 128), np.float16)
    enorm = np.empty((CORES, 128, EC // 128), np.float16)
    for c in range(CORES):
        eidx[c] = pidx[c].reshape(EC // 16, 16).T
        edl[c] = pdl[c].reshape(EC // 128, 128).T
        enorm[c] = pnorm[c].reshape(EC // 128, 128).T
    nself = np.ascontiguousarray(
        nself_flat.reshape(CORES, NB, 128).transpose(0, 2, 1))
    return T, eidx, edl, enorm, nself


def _host_fallback(x, edge_index, edge_weight, W1, b1, W2, b2, Wres, bres,
                   gamma1, beta1, gamma2, beta2):
    """Vectorized numpy reference (sort + reduceat); used only if the
    device path raises."""
    n = x.shape[0]
    loop = np.arange(n, dtype=np.int64)
    src = np.concatenate([edge_index[0].astype(np.int64), loop])
    dst = np.concatenate([edge_index[1].astype(np.int64), loop])
    w = np.concatenate([edge_weight, np.ones(n, np.float32)])

    deg = np.bincount(dst, weights=w, minlength=n).astype(np.float32)
    dinv = np.where(deg > 0, 1.0 / np.sqrt(np.maximum(deg, 1e-30)), 0.0)
    dinv = dinv.astype(np.float32)
    norm = dinv[src] * w * dinv[dst]
    order = np.argsort(dst, kind="stable")
    ssrc, sdst, snorm = src[order], dst[order], norm[order]
    starts = np.searchsorted(sdst, np.arange(n))

    def conv(h, W, b):
        hw = h @ W
        seg = np.add.reduceat(snorm[:, None] * hw[ssrc], starts, axis=0)
        seg[np.diff(np.concatenate([starts, [len(sdst)]])) == 0] = 0
        return seg + b

    def ln(v, g, be):
        mu = v.mean(-1, keepdims=True)
        var = np.square(v - mu).mean(-1, keepdims=True)
        return (v - mu) / np.sqrt(var + LN_EPS) * g + be

    h = ln(x @ Wres + bres + np.maximum(conv(x, W1, b1), 0), gamma1, beta1)
    h = ln(h + conv(h, W2, b2), gamma2, beta2)
    return h.astype(np.float32)


def kernel(x, edge_index, edge_weight, W1, b1, W2, b2, Wres, bres,
           gamma1, beta1, gamma2, beta2):
    global LAST_EXEC_NS
    import time as _time
    x = np.asarray(x, np.float32)
    edge_index = np.asarray(edge_index)
    edge_weight = np.asarray(edge_weight, np.float32)
    W1 = np.asarray(W1, np.float32)
    b1 = np.asarray(b1, np.float32)
    W2 = np.asarray(W2, np.float32)
    b2 = np.asarray(b2, np.float32)
    Wres = np.asarray(Wres, np.float32)
    bres = np.asarray(bres, np.float32)
    gamma1 = np.asarray(gamma1, np.float32)
    beta1 = np.asarray(beta1, np.float32)
    gamma2 = np.asarray(gamma2, np.float32)
    beta2 = np.asarray(beta2, np.float32)

    try:
        return _device_kernel(x, edge_index, edge_weight, W1, b1, W2, b2,
                              Wres, bres, gamma1, beta1, gamma2, beta2)
    except Exception:
        import traceback
        traceback.print_exc(file=sys.stderr)
        print("kernel: device path failed; using host fallback",
              file=sys.stderr)
        return _host_fallback(x, edge_index, edge_weight, W1, b1, W2, b2,
                              Wres, bres, gamma1, beta1, gamma2, beta2)


def _device_kernel(x, edge_index, edge_weight, W1, b1, W2, b2, Wres, bres,
                   gamma1, beta1, gamma2, beta2):
    global LAST_EXEC_NS
    import time as _time
    _t0 = _time.perf_counter()
    T, eidx, edl, enorm, nself = _preprocess(x, edge_index, edge_weight)
    _t1 = _time.perf_counter()

    flags = (
        bool(np.any(b1 != 0)), bool(np.any(bres != 0)), bool(np.any(b2 != 0)),
        bool(np.any(gamma1 != 1)), bool(np.any(beta1 != 0)),
        bool(np.any(gamma2 != 1)), bool(np.any(beta2 != 0)),
    )
    key = (T, flags, DBG_STAGE)
    if key not in _CACHE:
        nc_new = _build_nc(T, flags, DBG_STAGE)
        _CACHE[key] = (nc_new, _make_runner(nc_new))
    nc, runner = _CACHE[key]

    xpad = np.zeros((NPAD, DIN), np.float32)
    xpad[:N] = x
    x16 = xpad.astype(np.float16)

    W1h = W1.astype(np.float16)
    Wresh = Wres.astype(np.float16)
    W2a = W2[:128].astype(np.float16)
    W2b = W2[128:].astype(np.float16)

    use_names = []
    for name, used, arr in [("b1", flags[0], b1), ("bres", flags[1], bres),
                            ("b2", flags[2], b2), ("g1", flags[3], gamma1),
                            ("be1", flags[4], beta1), ("g2", flags[5], gamma2),
                            ("be2", flags[6], beta2)]:
        if used:
            use_names.append((name, np.broadcast_to(
                arr.astype(np.float32), (128, DH)).copy()))

    in_maps = []
    for c in range(CORES):
        m = {
            "xT": np.ascontiguousarray(x16[c * NPC:(c + 1) * NPC].T),
            "eidx": eidx[c], "edl": edl[c], "enorm": enorm[c],
            "nself": nself[c],
            "W1": W1h, "Wres": Wresh, "W2a": W2a, "W2b": W2b,
        }
        for name, arr in use_names:
            m[name] = arr
        in_maps.append(m)

    _t2 = _time.perf_counter()
    results = runner(in_maps)
    _t3 = _time.perf_counter()
    LAST_EXEC_NS = int((_t3 - _t2) * 1e9)
    if os.environ.get("KERNEL_TIMING"):
        print(f"[timing] preprocess={_t1-_t0:.3f}s inmaps={_t2-_t1:.3f}s "
              f"device_call={_t3-_t2:.3f}s", file=sys.stderr)

    out = np.concatenate([results[c]["out"] for c in range(CORES)], axis=0)
    return np.ascontiguousarray(out[:N]).astype(np.float32)


# revision 18
# speedup vs baseline: 34.3292x; 1.0111x over previous
"""GCNRouting2Hop Trainium2 kernel (8 NeuronCores, SPMD).

Strategy (graph/data parallel, dst-node partitioned):
  - Nodes padded 20000 -> 20480 = 8 cores x 20 blocks x 128 dst rows.
  - Host: sym-norm per edge, self loops handled separately (diagonal
    matmul against the locally-kept h block), sort remaining edges by
    dst, pad each 128-dst block's edge list to a uniform tile count T.
  - Device per core:
      h1 = x_local @ W1 (fp16)          -> AllGather -> h1_full in HBM
      conv1: per dst block, dma_gather h1_full[src] rows (fp16, <=1024
        idxs per call), build one-hot-x-norm selection tiles S on
        VectorE (cached + reused by conv2), segment-sum via PE matmul
        PSUM accumulation; epilogue relu/residual/LayerNorm.
      h2 = h' @ W2 (PE transpose + matmul) -> AllGather -> h2_full
      conv2: same aggregation (cached S); epilogue residual+LayerNorm.
  - Host: concat 8 x [2560, 256] -> [20000, 256] float32.
"""
import os
import sys

sys.path.insert(0, "/opt/trn_rl_repo")

import numpy as np

from concourse import bass, mybir, tile, bacc
from concourse import bass2jax
from concourse.masks import make_identity

CORES = 8
N = 20000
NPAD = 20480
NPC = NPAD // CORES        # 2560 nodes per core
NB = NPC // 128            # 20 dst blocks per core
DIN = 128
DH = 256
LN_EPS = 1e-5
GCH = 8                    # gather chunk: 8 tiles = 1024 idxs (SWDGE ring cap)

f32 = mybir.dt.float32
f16 = mybir.dt.float16
i16 = mybir.dt.int16

_CACHE = {}
LAST_EXEC_NS = None
TRACE = False
DBG_STAGE = os.environ.get("DBG_STAGE", "FULL")  # A | C1 | C1W2 | FULL


def _make_runner(nc):
    """Build a persistent jitted SPMD dispatcher for `nc` (trace once)."""
    import jax
    from jax.sharding import Mesh, PartitionSpec
    from jax.experimental.shard_map import shard_map

    bass2jax.install_neuronx_cc_hook()
    partition_name = (nc.partition_id_tensor.name
                      if nc.partition_id_tensor else None)
    in_names, out_names, out_avals, zero_shapes = [], [], [], []
    for alloc in nc.m.functions[0].allocations:
        if not isinstance(alloc, mybir.MemoryLocationSet):
            continue
        name = alloc.memorylocations[0].name
        if alloc.kind == "ExternalInput":
            if name != partition_name:
                in_names.append(name)
        elif alloc.kind == "ExternalOutput":
            shape = tuple(alloc.tensor_shape)
            dtype = mybir.dt.np(alloc.dtype)
            out_names.append(name)
            out_avals.append(jax.core.ShapedArray(shape, dtype))
            zero_shapes.append((shape, dtype))
    n_params = len(in_names)
    all_names = list(in_names)
    if partition_name is not None:
        all_names.append(partition_name)

    def _body(*args):
        operands = list(args)
        if partition_name is not None:
            operands.append(bass2jax.partition_id_tensor())
        outs = bass2jax._bass_exec_p.bind(
            *operands,
            out_avals=tuple(out_avals),
            in_names=tuple(all_names),
            out_names=tuple(out_names),
            lowering_input_output_aliases=(),
            sim_require_finite=True,
            sim_require_nnan=True,
            nc=nc,
        )
        return tuple(outs)

    devices = jax.devices()[:CORES]
    mesh = Mesh(np.asarray(devices), ("core",))
    n_outs = len(out_names)
    in_specs = (PartitionSpec("core"),) * n_params
    out_specs = (PartitionSpec("core"),) * n_outs
    sharded = jax.jit(
        shard_map(_body, mesh=mesh, in_specs=in_specs, out_specs=out_specs,
                  check_rep=False),
        keep_unused=True,
    )

    def run(in_maps):
        concat_in = [
            np.concatenate([np.asarray(m[name]) for m in in_maps], axis=0)
            for name in in_names
        ]
        out_arrs = sharded(*concat_in)
        return [
            {name: np.asarray(out_arrs[i]).reshape(CORES, *out_avals[i].shape)[c]
             for i, name in enumerate(out_names)}
            for c in range(CORES)
        ]

    return run


def _build_nc(T, flags, stage="FULL", sim1=False):
    """Build + compile the SPMD program. T = gather tiles per dst block."""
    use_b1, use_bres, use_b2, use_g1, use_be1, use_g2, use_be2 = flags
    EC = NB * T * 128          # non-self edges per core (padded)

    nc = bacc.Bacc("TRN2", target_bir_lowering=False, debug=False,
                   num_devices=1 if sim1 else CORES,
                   detect_race_conditions=False)

    xT_ext = nc.dram_tensor("xT", [DIN, NPC], f16, kind="ExternalInput")
    eidx_ext = nc.dram_tensor("eidx", [16, EC // 16], i16, kind="ExternalInput")
    edl_ext = nc.dram_tensor("edl", [128, EC // 128], f16, kind="ExternalInput")
    enorm_ext = nc.dram_tensor("enorm", [128, EC // 128], f16, kind="ExternalInput")
    nself_ext = nc.dram_tensor("nself", [128, NB], f32, kind="ExternalInput")
    W1_ext = nc.dram_tensor("W1", [DIN, DH], f16, kind="ExternalInput")
    Wres_ext = nc.dram_tensor("Wres", [DIN, DH], f16, kind="ExternalInput")
    W2a_ext = nc.dram_tensor("W2a", [128, DH], f16, kind="ExternalInput")
    W2b_ext = nc.dram_tensor("W2b", [128, DH], f16, kind="ExternalInput")
    vec_exts = {}
    for name, used in [("b1", use_b1), ("bres", use_bres), ("b2", use_b2),
                       ("g1", use_g1), ("be1", use_be1), ("g2", use_g2),
                       ("be2", use_be2)]:
        if used:
            vec_exts[name] = nc.dram_tensor(name, [128, DH], f32,
                                            kind="ExternalInput")
    out_ext = nc.dram_tensor("out", [NPC, DH], f16, kind="ExternalOutput")

    with tile.TileContext(nc) as tc:
        with (
            tc.tile_pool(name="dram", bufs=1, space="DRAM") as dram,
            tc.tile_pool(name="const", bufs=1) as const,
            tc.tile_pool(name="hp", bufs=1) as hpp,
            tc.tile_pool(name="hsb", bufs=1) as hsbp,
            tc.tile_pool(name="scache", bufs=1) as scp,
            tc.tile_pool(name="gp", bufs=1) as gp,
            tc.tile_pool(name="wk", bufs=1) as wk,
            tc.tile_pool(name="ps_mm", bufs=2, space="PSUM") as ps_mm,
            tc.tile_pool(name="ps_agg", bufs=2, space="PSUM") as ps_agg,
            tc.tile_pool(name="ps_tp", bufs=2, space="PSUM") as ps_tp,
        ):
            h1_shard = dram.tile([NPC, DH], f16)
            h2_shard = dram.tile([NPC, DH], f16)
            h1_full = dram.tile([NPAD, DH], f16, addr_space="Shared")
            h2_full = dram.tile([NPAD, DH], f16, addr_space="Shared")

            # ---- constants ----
            xT = const.tile([DIN, NPC], f16)
            nc.sync.dma_start(xT[:], xT_ext[:])
            eidx = const.tile([128, EC // 16], i16)
            for k in range(8):
                nc.sync.dma_start(eidx[16 * k:16 * (k + 1), :], eidx_ext[:])
            edl16 = const.tile([128, EC // 128], f16)
            nc.sync.dma_start(edl16[:], edl_ext[:])
            edl = const.tile([128, EC // 128], f32)
            nc.vector.tensor_copy(edl[:], edl16[:])
            enorm16 = const.tile([128, EC // 128], f16)
            nc.sync.dma_start(enorm16[:], enorm_ext[:])
            enorm = const.tile([128, EC // 128], f32)
            nc.vector.tensor_copy(enorm[:], enorm16[:])
            nself = const.tile([128, NB], f32)
            nc.sync.dma_start(nself[:], nself_ext[:])
            W1sb = const.tile([DIN, DH], f16)
            nc.sync.dma_start(W1sb[:], W1_ext[:])
            Wressb = const.tile([DIN, DH], f16)
            nc.sync.dma_start(Wressb[:], Wres_ext[:])
            W2asb = const.tile([128, DH], f16)
            nc.sync.dma_start(W2asb[:], W2a_ext[:])
            W2bsb = const.tile([128, DH], f16)
            nc.sync.dma_start(W2bsb[:], W2b_ext[:])
            vecs = {}
            for name in vec_exts:
                v = const.tile([128, DH], f32, name=f"vec_{name}")
                nc.sync.dma_start(v[:], vec_exts[name][:])
                vecs[name] = v

            iota_i = const.tile([128, 128], i16)
            nc.gpsimd.iota(iota_i[:], pattern=[[1, 128]], base=0,
                           channel_multiplier=0)
            iota_f = const.tile([128, 128], f16)
            nc.vector.tensor_copy(iota_f[:], iota_i[:])
            iotac_i = const.tile([128, 1], i16)
            nc.gpsimd.iota(iotac_i[:], pattern=[[1, 1]], base=0,
                           channel_multiplier=1)
            iotac_f = const.tile([128, 1], f32)
            nc.vector.tensor_copy(iotac_f[:], iotac_i[:])
            ident = const.tile([128, 128], f32)
            make_identity(nc, ident[:])
            epsb = const.tile([128, 1], f32)
            nc.vector.memset(epsb[:], LN_EPS)

            # ---- stage A: h1 shard = x_local @ W1, then AllGather ----
            h1sb_tiles = []
            for nb in range(NB):
                ps = ps_mm.tile([128, DH], f32, tag="mm", name=f"h1ps{nb}")
                nc.tensor.matmul(ps[:], xT[:, nb * 128:(nb + 1) * 128],
                                 W1sb[:], start=True, stop=True)
                h1sb = hsbp.tile([128, DH], f16, tag=f"h1sb{nb}",
                                 name=f"h1sb{nb}")
                nc.scalar.activation(h1sb[:], ps[:],
                                     mybir.ActivationFunctionType.Copy)
                h1sb_tiles.append(h1sb)
                nc.sync.dma_start(h1_shard[nb * 128:(nb + 1) * 128, :], h1sb[:])

            if sim1:
                nc.sync.dma_start(h1_full[0:NPC, :], h1_shard[:])
            else:
                nc.gpsimd.collective_compute(
                    "AllGather", mybir.AluOpType.bypass,
                    replica_groups=[list(range(CORES))],
                    ins=[h1_shard[:].opt()], outs=[h1_full[:].opt()],
                )

            # ---- cached selection tiles (built once, used by both convs) --
            s_tiles = {}

            def get_S(nb, t):
                key = (nb, t)
                if key not in s_tiles:
                    col = nb * T + t
                    S = scp.tile([128, 128], f16, tag=f"S{nb}_{t}",
                                 name=f"S{nb}_{t}")
                    nc.vector.tensor_scalar(
                        S[:], iota_f[:], edl[:, col:col + 1],
                        enorm[:, col:col + 1],
                        mybir.AluOpType.is_equal, mybir.AluOpType.mult,
                    )
                    s_tiles[key] = S
                return s_tiles[key]

            def get_Sself(nb):
                key = ("self", nb)
                if key not in s_tiles:
                    S = scp.tile([128, 128], f16, tag=f"Ss{nb}",
                                 name=f"Ss{nb}")
                    nc.vector.tensor_scalar(
                        S[:], iota_f[:], iotac_f[:], nself[:, nb:nb + 1],
                        mybir.AluOpType.is_equal, mybir.AluOpType.mult,
                    )
                    s_tiles[key] = S
                return s_tiles[key]

            # Gather chunks span block boundaries: the global tile stream
            # (NB*T tiles per conv) is fetched in uniform GCH-tile calls.
            conv_gb = {}

            def get_gb(g, src_full, conv):
                """Return (tile, slot) holding global tile g of this conv."""
                cki = g // GCH
                key = (conv, cki)
                if key not in conv_gb:
                    t0g = cki * GCH
                    tn = min(GCH, NB * T - t0g)
                    gb = gp.tile([128, GCH, DH], f16, tag="g", bufs=4,
                                 name=f"g{conv}_{cki}")
                    nc.gpsimd.dma_gather(
                        out_ap=gb[:, :tn, :], in_ap=src_full[:],
                        idxs_ap=eidx[:, t0g * 8:(t0g + tn) * 8],
                        num_idxs=tn * 128, num_idxs_reg=tn * 128,
                        elem_size=DH,
                    )
                    conv_gb[key] = gb
                return conv_gb[key], g % GCH

            def conv_block(nb, src_full, conv, hsb_tile):
                """Aggregate one dst block; returns PSUM tile with the sum."""
                aps = ps_agg.tile([128, DH], f32, tag="agg",
                                  name=f"agg{conv}_{nb}")
                # self-loop contribution from the locally-kept h block
                nc.tensor.matmul(aps[:], get_Sself(nb)[:], hsb_tile[:],
                                 start=True, stop=False)
                for t in range(T):
                    gb, slot = get_gb(nb * T + t, src_full, conv)
                    S = get_S(nb, t)
                    nc.tensor.matmul(aps[:], S[:], gb[:, slot, :],
                                     start=False, stop=(t == T - 1))
                return aps

            def layer_norm(t1, gkey, bekey, out_tile, pfx):
                """out = LN(t1) via E[x^2] - mu^2 (fused final scale)."""
                r1 = wk.tile([128, 1], f32, tag="r1", bufs=2, name=f"{pfx}r1")
                nc.vector.tensor_reduce(r1[:], t1[:], mybir.AxisListType.X,
                                        mybir.AluOpType.add)
                scr = wk.tile([128, DH], f16, tag="scr", bufs=2,
                              name=f"{pfx}scr")
                r2 = wk.tile([128, 1], f32, tag="r2", bufs=2, name=f"{pfx}r2")
                nc.scalar.activation(scr[:], t1[:],
                                     mybir.ActivationFunctionType.Square,
                                     accum_out=r2[:])
                mu = wk.tile([128, 1], f32, tag="mu", bufs=2, name=f"{pfx}mu")
                nc.vector.tensor_scalar_mul(mu[:], r1[:], 1.0 / DH)
                # bias for Sqrt: eps - mu^2
                bs = wk.tile([128, 1], f32, tag="bs", bufs=2, name=f"{pfx}bs")
                nc.vector.tensor_scalar(bs[:], mu[:], mu[:, :1], -1.0,
                                        mybir.AluOpType.mult,
                                        mybir.AluOpType.mult)
                nc.vector.tensor_scalar_add(bs[:], bs[:], LN_EPS)
                std = wk.tile([128, 1], f32, tag="std", bufs=2,
                              name=f"{pfx}std")
                nc.scalar.activation(std[:], r2[:],
                                     mybir.ActivationFunctionType.Sqrt,
                                     bias=bs[:], scale=1.0 / DH)
                rstd = wk.tile([128, 1], f32, tag="rstd", bufs=2,
                               name=f"{pfx}rstd")
                nc.vector.reciprocal(rstd[:], std[:])
                nc.vector.tensor_scalar(out_tile[:], t1[:], mu[:, :1],
                                        rstd[:, :1],
                                        mybir.AluOpType.subtract,
                                        mybir.AluOpType.mult)
                if gkey is not None:
                    nc.vector.tensor_tensor(out_tile[:], out_tile[:],
                                            vecs[gkey][:],
                                            mybir.AluOpType.mult)
                if bekey is not None:
                    nc.vector.tensor_tensor(out_tile[:], out_tile[:],
                                            vecs[bekey][:],
                                            mybir.AluOpType.add)

            # ---- conv1 + epilogue + W2 matmul ----
            hp_tiles = []
            h2sb_tiles = []
            for nb in range(NB if stage != "A" else 0):
                aps = conv_block(nb, h1_full, 1, h1sb_tiles[nb])
                # t0 = relu(agg + b1)
                t0 = wk.tile([128, DH], f32, tag="t0", bufs=2, name=f"t0_{nb}")
                if use_b1:
                    nc.vector.tensor_tensor(t0[:], aps[:], vecs["b1"][:],
                                            mybir.AluOpType.add)
                    nc.vector.tensor_scalar_max(t0[:], t0[:], 0.0)
                else:
                    nc.scalar.activation(t0[:], aps[:],
                                         mybir.ActivationFunctionType.Relu)
                # residual: rps = x_block @ Wres (+ bres)
                rps = ps_mm.tile([128, DH], f32, tag="mm", name=f"rps{nb}")
                nc.tensor.matmul(rps[:], xT[:, nb * 128:(nb + 1) * 128],
                                 Wressb[:], start=True, stop=True)
                t1 = wk.tile([128, DH], f32, tag="t1", bufs=2, name=f"t1_{nb}")
                nc.vector.tensor_tensor(t1[:], t0[:], rps[:],
                                        mybir.AluOpType.add)
                if use_bres:
                    nc.vector.tensor_tensor(t1[:], t1[:], vecs["bres"][:],
                                            mybir.AluOpType.add)
                hp = hpp.tile([128, DH], f32, tag=f"hp{nb}", name=f"hp{nb}")
                layer_norm(t1, "g1" if use_g1 else None,
                           "be1" if use_be1 else None, hp, f"c1_{nb}_")
                hp_tiles.append(hp)
                if stage == "C1":
                    nc.sync.dma_start(out_ext[nb * 128:(nb + 1) * 128, :],
                                      hp[:])
                    continue
                # transpose h' block and compute h2 = h' @ W2
                h2ps = ps_mm.tile([128, DH], f32, tag="mm", name=f"h2ps{nb}")
                for half in range(2):
                    tp = ps_tp.tile([128, 128], f32, tag="tp",
                                    name=f"tp{nb}_{half}")
                    nc.tensor.transpose(tp[:],
                                        hp[:, half * 128:(half + 1) * 128],
                                        ident[:])
                    hT = wk.tile([128, 128], f16, tag="hT", bufs=3,
                                 name=f"hT{nb}_{half}")
                    nc.scalar.activation(hT[:], tp[:],
                                         mybir.ActivationFunctionType.Copy)
                    nc.tensor.matmul(h2ps[:], hT[:],
                                     W2asb[:] if half == 0 else W2bsb[:],
                                     start=(half == 0), stop=(half == 1))
                h2sb = hsbp.tile([128, DH], f16, tag=f"h2sb{nb}",
                                 name=f"h2sb{nb}")
                nc.scalar.activation(h2sb[:], h2ps[:],
                                     mybir.ActivationFunctionType.Copy)
                h2sb_tiles.append(h2sb)
                nc.sync.dma_start(h2_shard[nb * 128:(nb + 1) * 128, :],
                                  h2sb[:])

            if stage in ("C1W2", "FULL"):
                if sim1:
                    nc.sync.dma_start(h2_full[0:NPC, :], h2_shard[:])
                else:
                    nc.gpsimd.collective_compute(
                        "AllGather", mybir.AluOpType.bypass,
                        replica_groups=[list(range(CORES))],
                        ins=[h2_shard[:].opt()], outs=[h2_full[:].opt()],
                    )

            # ---- conv2 + epilogue ----
            for nb in range(NB if stage == "FULL" else 0):
                aps = conv_block(nb, h2_full, 2, h2sb_tiles[nb])
                t1 = wk.tile([128, DH], f32, tag="t1", bufs=2,
                             name=f"c2t1_{nb}")
                nc.vector.tensor_tensor(t1[:], aps[:], hp_tiles[nb][:],
                                        mybir.AluOpType.add)
                if use_b2:
                    nc.vector.tensor_tensor(t1[:], t1[:], vecs["b2"][:],
                                            mybir.AluOpType.add)
                ot = wk.tile([128, DH], f16, tag="ot", bufs=3, name=f"ot_{nb}")
                layer_norm(t1, "g2" if use_g2 else None,
                           "be2" if use_be2 else None, ot, f"c2_{nb}_")
                nc.sync.dma_start(out_ext[nb * 128:(nb + 1) * 128, :], ot[:])

    nc.compile()
    return nc


def _preprocess(x, edge_index, edge_weight):
    """Host-side: sym norm, self-loop coeffs, dst sort, block padding."""
    src = np.ascontiguousarray(edge_index[0]).astype(np.int64)
    dst = np.ascontiguousarray(edge_index[1]).astype(np.int64)
    w = edge_weight.astype(np.float32)
    n = x.shape[0]

    # self-loop weight 1 is added to every node's in-degree
    deg = (np.bincount(dst, weights=w, minlength=n) + 1.0).astype(np.float32)
    dinv = (1.0 / np.sqrt(deg)).astype(np.float32)
    norm = dinv[src] * w * dinv[dst]
    nself_flat = np.zeros(NPAD, np.float32)
    nself_flat[:n] = dinv * dinv

    # group by 128-dst block only (intra-block order is irrelevant to the
    # one-hot segment-sum); int16 keys hit numpy's radix sort (~12x faster)
    nblocks_total = NPAD // 128  # 160
    order = np.argsort((dst >> 7).astype(np.int16), kind="stable")
    ssrc = src[order].astype(np.int16)
    sdst = dst[order]
    snorm = norm[order]
    blk = (sdst >> 7).astype(np.int64)
    cnt = np.bincount(blk, minlength=nblocks_total)
    T = int(np.max((cnt + 127) // 128))
    EC = NB * T * 128

    pidx = np.zeros((CORES, EC), np.int16)
    pnorm = np.zeros((CORES, EC), np.float32)
    pdl = np.zeros((CORES, EC), np.float32)
    off = np.concatenate([[0], np.cumsum(cnt)])
    for b in range(nblocks_total):
        c, bl = divmod(b, NB)
        s, e = off[b], off[b + 1]
        k = bl * T * 128
        m = e - s
        pidx[c, k:k + m] = ssrc[s:e]
        pnorm[c, k:k + m] = snorm[s:e]
        pdl[c, k:k + m] = (sdst[s:e] & 127).astype(np.float32)

    eidx = np.empty((CORES, 16, EC // 16), np.int16)
    edl = np.empty((CORES, 128, EC // 128), np.float16)
    enorm = np.empty((CORES, 128, EC // 128), np.float16)
    for c in range(CORES):
        eidx[c] = pidx[c].reshape(EC // 16, 16).T
        edl[c] = pdl[c].reshape(EC // 128, 128).T
        enorm[c] = pnorm[c].reshape(EC // 128, 128).T
    nself = np.ascontiguousarray(
        nself_flat.reshape(CORES, NB, 128).transpose(0, 2, 1))
    return T, eidx, edl, enorm, nself


def _host_fallback(x, edge_index, edge_weight, W1, b1, W2, b2, Wres, bres,
                   gamma1, beta1, gamma2, beta2):
    """Vectorized numpy reference (sort + reduceat); used only if the
    device path raises."""
    n = x.shape[0]
    loop = np.arange(n, dtype=np.int64)
    src = np.concatenate([edge_index[0].astype(np.int64), loop])
    dst = np.concatenate([edge_index[1].astype(np.int64), loop])
    w = np.concatenate([edge_weight, np.ones(n, np.float32)])

    deg = np.bincount(dst, weights=w, minlength=n).astype(np.float32)
    dinv = np.where(deg > 0, 1.0 / np.sqrt(np.maximum(deg, 1e-30)), 0.0)
    dinv = dinv.astype(np.float32)
    norm = dinv[src] * w * dinv[dst]
    order = np.argsort(dst, kind="stable")
    ssrc, sdst, snorm = src[order], dst[order], norm[order]
    starts = np.searchsorted(sdst, np.arange(n))

    def conv(h, W, b):
        hw = h @ W
        seg = np.add.reduceat(snorm[:, None] * hw[ssrc], starts, axis=0)
        seg[np.diff(np.concatenate([starts, [len(sdst)]])) == 0] = 0
        return seg + b

    def ln(v, g, be):
        mu = v.mean(-1, keepdims=True)
        var = np.square(v - mu).mean(-1, keepdims=True)
        return (v - mu) / np.sqrt(var + LN_EPS) * g + be

    h = ln(x @ Wres + bres + np.maximum(conv(x, W1, b1), 0), gamma1, beta1)
    h = ln(h + conv(h, W2, b2), gamma2, beta2)
    return h.astype(np.float32)


def kernel(x, edge_index, edge_weight, W1, b1, W2, b2, Wres, bres,
           gamma1, beta1, gamma2, beta2):
    global LAST_EXEC_NS
    import time as _time
    x = np.asarray(x, np.float32)
    edge_index = np.asarray(edge_index)
    edge_weight = np.asarray(edge_weight, np.float32)
    W1 = np.asarray(W1, np.float32)
    b1 = np.asarray(b1, np.float32)
    W2 = np.asarray(W2, np.float32)
    b2 = np.asarray(b2, np.float32)
    Wres = np.asarray(Wres, np.float32)
    bres = np.asarray(bres, np.float32)
    gamma1 = np.asarray(gamma1, np.float32)
    beta1 = np.asarray(beta1, np.float32)
    gamma2 = np.asarray(gamma2, np.float32)
    beta2 = np.asarray(beta2, np.float32)

    try:
        return _device_kernel(x, edge_index, edge_weight, W1, b1, W2, b2,
                              Wres, bres, gamma1, beta1, gamma2, beta2)
    except Exception:
        import traceback
        traceback.print_exc(file=sys.stderr)
        print("kernel: device path failed; using host fallback",
              file=sys.stderr)
        return _host_fallback(x, edge_index, edge_weight, W1, b1, W2, b2,
                              Wres, bres, gamma1, beta1, gamma2, beta2)


def _device_kernel(x, edge_index, edge_weight, W1, b1, W2, b2, Wres, bres,
                   gamma1, beta1, gamma2, beta2):
    global LAST_EXEC_NS
    import time as _time
    _t0 = _time.perf_counter()
    T, eidx, edl, enorm, nself = _preprocess(x, edge_index, edge_weight)
    _t1 = _time.perf_counter()

    flags = (
        bool(np.any(b1 != 0)), bool(np.any(bres != 0)), bool(np.any(b2 != 0)),
        bool(np.any(gamma1 != 1)), bool(np.any(beta1 != 0)),
        bool(np.any(gamma2 != 1)), bool(np.any(beta2 != 0)),
    )
    key = (T, flags, DBG_STAGE)
    if key not in _CACHE:
        nc_new = _build_nc(T, flags, DBG_STAGE)
        _CACHE[key] = (nc_new, _make_runner(nc_new))
    nc, runner = _CACHE[key]

    xpad = np.zeros((NPAD, DIN), np.float32)
    xpad[:N] = x
    x16 = xpad.astype(np.float16)

    W1h = W1.astype(np.float16)
    Wresh = Wres.astype(np.float16)
    W2a = W2[:128].astype(np.float16)
    W2b = W2[128:].astype(np.float16)

    use_names = []
    for name, used, arr in [("b1", flags[0], b1), ("bres", flags[1], bres),
                            ("b2", flags[2], b2), ("g1", flags[3], gamma1),
                            ("be1", flags[4], beta1), ("g2", flags[5], gamma2),
                            ("be2", flags[6], beta2)]:
        if used:
            use_names.append((name, np.broadcast_to(
                arr.astype(np.float32), (128, DH)).copy()))

    in_maps = []
    for c in range(CORES):
        m = {
            "xT": np.ascontiguousarray(x16[c * NPC:(c + 1) * NPC].T),
            "eidx": eidx[c], "edl": edl[c], "enorm": enorm[c],
            "nself": nself[c],
            "W1": W1h, "Wres": Wresh, "W2a": W2a, "W2b": W2b,
        }
        for name, arr in use_names:
            m[name] = arr
        in_maps.append(m)

    _t2 = _time.perf_counter()
    results = runner(in_maps)
    _t3 = _time.perf_counter()
    LAST_EXEC_NS = int((_t3 - _t2) * 1e9)
    if os.environ.get("KERNEL_TIMING"):
        print(f"[timing] preprocess={_t1-_t0:.3f}s inmaps={_t2-_t1:.3f}s "
              f"device_call={_t3-_t2:.3f}s", file=sys.stderr)

    out = np.concatenate([results[c]["out"] for c in range(CORES)], axis=0)
    return np.ascontiguousarray(out[:N]).astype(np.float32)
